# revision 1
# baseline (speedup 1.0000x reference)
"""Trainium2 Bass kernel for nn_CenterAgent (scatter_memory).

Self-contained: takes FULL inputs (B=256), shards batch across 8 NeuronCores
(pure data parallel, 32 samples/core), runs one Bass/Tile program per core via
run_bass_kernel_spmd, gathers the full [256, 24] output.

Algorithm restructuring (per core, all matmuls float32r on the PE at
1 cycle/row):
  - bilinear 7x7->28x28 resize + conv1 over the 512 feature channels is
    decomposed as (a) a 512->128 channel contraction per conv tap at 7x7
    resolution, (b) an "upsample-and-sum" matmul against constant matrices
    U_t = Ashift_di (x) Ashift_dj, with tap pairs stacked on partition halves.
  - the scatter (center map) runs on GPSIMD local_scatter with last-write-wins
    duplicate resolution done on the vector engine (100x100 compare + mask).
  - image + center-map channels of conv1 go through a DRAM-padded scratch and
    a per-sample [36, 784] im2col that folds into conv1's PSUM accumulation.
  - conv2/3/4 are 9-tap shifted-window matmuls with samples packed via column
    tiling / block-diagonal weights; fc1 consumes a PE-transposed conv4 output.
"""

import os
from contextlib import ExitStack

import ml_dtypes
import numpy as np

import concourse.bass as bass
import concourse.tile as tile
from concourse import bacc, mybir
from concourse.bass_utils import run_bass_kernel_spmd

NCORES = 8
B = 256
BL = B // NCORES  # 32 samples per core
SC = 8            # samples per pipeline chunk
F32 = mybir.dt.float32
F32R = mybir.dt.float32r
F16 = mybir.dt.float16
BF16 = mybir.dt.bfloat16
I16 = mybir.dt.int16
ALU = mybir.AluOpType
ACTF = mybir.ActivationFunctionType

EVEN = [0, 2, 4, 6]
ODD = [1, 3, 5, 7]


# ----------------------------------------------------------------- host consts

def _bilinear_A():
    A = np.zeros((28, 7), np.float32)
    for i in range(28):
        t = (i + 0.5) / 4 - 0.5
        p0 = int(np.floor(t))
        w = t - p0
        A[i, min(max(p0, 0), 6)] += 1 - w
        A[i, min(max(p0 + 1, 0), 6)] += w
    return A


def _shifted_A(d):
    A = _bilinear_A()
    S = np.zeros_like(A)
    for i in range(28):
        src = i + d - 1
        if 0 <= src < 28:
            S[i] = A[src]
    return S


def _utap(t):
    di, dj = divmod(t, 3)
    return np.einsum(
        "ip,jq->pqij", _shifted_A(di), _shifted_A(dj)
    ).reshape(49, 784).astype(np.float32)


def _build_consts(w1, b1, w2, b2, w3, b3, w4, b4, fw1, fb1, fw2, fb2):
    w1 = np.asarray(w1, np.float32)
    w1f = w1[:, 3:515]  # [128o, 512c, 3, 3]
    c_w1fe = np.zeros((128, 4, 512), np.float32)
    c_w1fo = np.zeros((128, 4, 512), np.float32)
    c_w1f8 = np.zeros((128, 4, 128), np.float32)
    for kb in range(4):
        blk = w1f[:, kb * 128:(kb + 1) * 128]  # [o, c, 3, 3]
        for ti, t in enumerate(EVEN):
            di, dj = divmod(t, 3)
            c_w1fe[:, kb, ti * 128:(ti + 1) * 128] = blk[:, :, di, dj].T
        for ti, t in enumerate(ODD):
            di, dj = divmod(t, 3)
            c_w1fo[:, kb, ti * 128:(ti + 1) * 128] = blk[:, :, di, dj].T
        c_w1f8[:, kb, :] = blk[:, :, 2, 2].T

    c_uall = np.zeros((128, 5, 784), np.float32)
    for t in range(4):
        c_uall[0:49, t] = _utap(EVEN[t])
        c_uall[64:113, t] = _utap(ODD[t])
    # tap 8: stage-0 splits its K-blocks across both partition halves; the
    # pair-4 contraction re-merges them, so U8 appears on both halves.
    c_uall[0:49, 4] = _utap(8)
    c_uall[64:113, 4] = _utap(8)

    chmap = [0, 1, 2, 515]
    c_w1ic3 = np.zeros((12, 3, 128), np.float32)
    for di in range(3):
        for ch in range(4):
            for dj in range(3):
                c_w1ic3[di * 4 + ch, dj] = w1[:, chmap[ch], di, dj]

    w2 = np.asarray(w2, np.float32)
    c_w2 = np.zeros((128, 9, 64), np.float32)
    for t in range(9):
        di, dj = divmod(t, 3)
        c_w2[:, t, :] = w2[:, :, di, dj].T  # [128c, 64o]

    w3 = np.asarray(w3, np.float32)
    c_w3 = np.zeros((128, 9, 64), np.float32)
    for t in range(9):
        di, dj = divmod(t, 3)
        wt = w3[:, :, di, dj].T  # [64c, 32o]
        c_w3[0:64, t, 0:32] = wt
        c_w3[64:128, t, 32:64] = wt

    w4 = np.asarray(w4, np.float32)
    c_w4 = np.zeros((128, 9, 64), np.float32)
    for t in range(9):
        di, dj = divmod(t, 3)
        wt = w4[:, :, di, dj].T  # [32c, 16o]
        for bi in range(4):
            c_w4[32 * bi:32 * bi + 32, t, 16 * bi:16 * bi + 16] = wt

    f3 = np.asarray(fw1, np.float32).reshape(256, 16, 784)
    c_fw1 = np.zeros((112, 7, 16, 256), np.float16)
    for c in range(7):
        c_fw1[:, c] = f3[:, :, 112 * c:112 * (c + 1)].transpose(2, 1, 0)

    fw2 = np.asarray(fw2, np.float32)  # [24, 256]
    c_fw2 = np.zeros((128, 2, 24), np.float32)
    c_fw2[:, 0] = fw2[:, 0:128].T
    c_fw2[:, 1] = fw2[:, 128:256].T

    ident2 = np.zeros((128, 64), np.float32)
    ident2[0:64] = np.eye(64)
    ident2[64:128] = np.eye(64)

    tri = np.triu(np.ones((100, 100), np.float32), k=1)
    c_tri = np.broadcast_to(tri, (32, 100, 100)).astype(np.float16)

    return {
        "c_w1fe": c_w1fe.astype(ml_dtypes.bfloat16),
        "c_w1fo": c_w1fo.astype(ml_dtypes.bfloat16),
        "c_w1f8": c_w1f8.astype(ml_dtypes.bfloat16),
        "c_uall": c_uall.astype(ml_dtypes.bfloat16),
        "c_w1ic3": c_w1ic3,
        "c_w2": c_w2.astype(ml_dtypes.bfloat16),
        "c_w3": c_w3.astype(ml_dtypes.bfloat16),
        "c_w4": c_w4.astype(ml_dtypes.bfloat16),
        "c_b1": np.asarray(b1, np.float32).reshape(128, 1),
        "c_b2": np.tile(np.asarray(b2, np.float32), 2).reshape(128, 1),
        "c_b3": np.tile(np.asarray(b3, np.float32), 4).reshape(128, 1),
        "c_b4": np.tile(np.asarray(b4, np.float32), 8).reshape(128, 1),
        "c_fw1": c_fw1,
        "c_fb1": np.asarray(fb1, np.float32).reshape(1, 256),
        "c_fw2": c_fw2,
        "c_fb2": np.asarray(fb2, np.float32).reshape(1, 24),
        "c_ident": ident2,
        "c_tri": c_tri,
        "c_k27": np.broadcast_to(np.arange(1, 28, dtype=np.float32), (32, 27)).copy(),
    }


_CONST_SPECS = {
    "c_w1fe": ([128, 4, 512], BF16),
    "c_w1fo": ([128, 4, 512], BF16),
    "c_w1f8": ([128, 4, 128], BF16),
    "c_uall": ([128, 5, 784], BF16),
    "c_w1ic3": ([12, 3, 128], F32R),
    "c_w2": ([128, 9, 64], BF16),
    "c_w3": ([128, 9, 64], BF16),
    "c_w4": ([128, 9, 64], BF16),
    "c_b1": ([128, 1], F32),
    "c_b2": ([128, 1], F32),
    "c_b3": ([128, 1], F32),
    "c_b4": ([128, 1], F32),
    "c_fw1": ([112, 7, 16, 256], F16),
    "c_fb1": ([1, 256], F32R),
    "c_fw2": ([128, 2, 24], F32R),
    "c_fb2": ([1, 24], F32R),
    "c_ident": ([128, 64], F32),
    "c_tri": ([32, 100, 100], F16),
    "c_k27": ([32, 27], F32),
}


# ------------------------------------------------------------------ device IR



def build_nc(reps=1):
    nc = bacc.Bacc("TRN2", target_bir_lowering=False, debug=False)
    image = nc.dram_tensor("image", [BL, 3, 28, 28], F32R, kind="ExternalInput").ap()
    features = nc.dram_tensor("features", [BL, 512, 7, 7], F32R, kind="ExternalInput").ap()
    centers = nc.dram_tensor("centers", [BL, 100, 4], F32, kind="ExternalInput").ap()
    cst = {
        name: nc.dram_tensor(name, shape, dt, kind="ExternalInput").ap()
        for name, (shape, dt) in _CONST_SPECS.items()
    }
    out_d = nc.dram_tensor("out", [BL, 24], F32, kind="ExternalOutput").ap()
    scratch = nc.dram_tensor("scratch", [BL, 4, 30, 30], F32R, kind="Internal").ap()

    with tile.TileContext(nc) as tc, ExitStack() as ctx:
        # ------------------------------------------------ constant tiles
        cp = ctx.enter_context(tc.tile_pool(name="consts", bufs=1))
        ct = {}
        for name, (shape, dt) in _CONST_SPECS.items():
            if name in ("c_tri", "c_k27", "c_fw1"):
                continue
            ct[name] = cp.tile(shape, dt, tag=name, name=name)
        ones32 = cp.tile([1, 32], F32R, tag="ones32")
        nc.vector.memset(ones32[:].bitcast(F32), 1.0)

        for _rep in range(reps):
          with ExitStack() as rctx:
              # ------------------------------------------------ scatter (center map)
              with tc.tile_pool(name="scat", bufs=1) as sp:
                  tri = sp.tile([32, 100, 100], F16, tag="tri")
                  nc.sync.dma_start(out=tri[:], in_=cst["c_tri"])
                  cen = sp.tile([32, 100, 4], F32, tag="cen")
                  nc.sync.dma_start(out=cen[:], in_=centers)

                  def fl(name):
                      return sp.tile([32, 100], F32, tag=name, name=name)

                  # floor(v) for v in [0, 28): sum_k 1[v >= k], k = 1..27
                  k27 = sp.tile([32, 27], F32, tag="k27")
                  nc.sync.dma_start(out=k27[:], in_=cst["c_k27"])
                  ge = sp.tile([32, 100, 27], F32, tag="ge")

                  def floor28(dst, coord_ap, name):
                      v = fl(name)
                      nc.vector.tensor_scalar_mul(v[:], coord_ap, 28.0)
                      nc.vector.tensor_tensor(
                          ge[:],
                          v[:].unsqueeze(2).broadcast_to([32, 100, 27]),
                          k27[:].unsqueeze(1).broadcast_to([32, 100, 27]),
                          ALU.is_ge,
                      )
                      nc.vector.tensor_reduce(dst[:], ge[:], mybir.AxisListType.X, ALU.add)

                  xp = fl("xp")
                  floor28(xp, cen[:, :, 0], "xs")
                  yp = fl("yp")
                  floor28(yp, cen[:, :, 1], "ys")
                  flat = fl("flat")
                  # flat = (yp*30 + xp) + 31  (padded 30x30 index)
                  nc.vector.scalar_tensor_tensor(flat[:], yp[:], 30.0, xp[:], ALU.mult, ALU.add)
                  nc.vector.tensor_scalar_add(flat[:], flat[:], 31.0)

                  flat16 = sp.tile([32, 100], F16, tag="flat16")
                  nc.vector.tensor_copy(flat16[:], flat[:])
                  D = sp.tile([32, 100, 100], F16, tag="D")
                  a0 = flat16[:].unsqueeze(2).broadcast_to([32, 100, 100])
                  a1 = flat16[:].unsqueeze(1).broadcast_to([32, 100, 100])
                  nc.vector.tensor_tensor(D[:], a0, a1, ALU.is_equal)
                  E = sp.tile([32, 100, 100], F16, tag="E")
                  nc.vector.tensor_mul(E[:], D[:], tri[:])
                  later = sp.tile([32, 100], F16, tag="later")
                  nc.vector.tensor_reduce(later[:], E[:], mybir.AxisListType.X, ALU.max)
                  lateri = sp.tile([32, 100], mybir.dt.uint8, tag="lateri")
                  nc.vector.tensor_copy(lateri[:], later[:])
                  neg1 = fl("neg1")
                  nc.vector.memset(neg1[:], -1.0)
                  idxf = fl("idxf")
                  nc.vector.select(idxf[:], lateri[:], neg1[:], flat[:])
                  idx16 = sp.tile([32, 100], I16, tag="idx16")
                  nc.vector.tensor_copy(idx16[:], idxf[:])
                  conf16 = sp.tile([32, 100], F16, tag="conf16")
                  nc.vector.tensor_copy(conf16[:], cen[:, :, 3])
                  cmap16 = sp.tile([32, 900], F16, tag="cmap16")
                  nc.gpsimd.local_scatter(cmap16[:], conf16[:], idx16[:],
                                          channels=32, num_elems=900, num_idxs=100)
                  cmap32 = sp.tile([32, 30, 30], F32R, tag="cmap32")
                  nc.vector.tensor_copy(cmap32[:], cmap16[:].rearrange("p (a b) -> p a b", a=30))
                  nc.sync.dma_start(out=scratch[:, 3], in_=cmap32[:])

                  # image zero-pad to scratch
                  ipad = sp.tile([96, 30, 30], F32R, tag="ipad")
                  nc.gpsimd.memset(ipad[:].bitcast(F32), 0.0)
                  nc.sync.dma_start(out=ipad[:, 1:29, 1:29],
                                    in_=image.rearrange("s c h w -> (s c) h w"))
                  nc.sync.dma_start(out=scratch[:, 0:3], in_=ipad[:])

              # persistent zero-padded buffers (memset once; interiors overwritten)
              for name in ct:
                  nc.sync.dma_start(out=ct[name][:], in_=cst[name])

              fwp = rctx.enter_context(tc.tile_pool(name="fw1", bufs=1))
              c_fw1_t = fwp.tile([112, 7, 16, 256], F16, tag="c_fw1", name="c_fw1_t")

              pp = rctx.enter_context(tc.tile_pool(name="persist", bufs=1))
              NSF = 4
              # bf16 stationary features, zero-padded cols 49-63 (M=64/group)
              fbuf = pp.tile([128, NSF, 4, 64], BF16, tag="fbuf")
              nc.gpsimd.memset(fbuf[:], 0.0)
              x1buf = pp.tile([128, SC, 30, 30], BF16, tag="x1buf")
              nc.gpsimd.memset(x1buf[:], 0.0)
              x2buf = pp.tile([128, 4, 30, 30], BF16, tag="x2buf")
              nc.gpsimd.memset(x2buf[:], 0.0)
              x3buf = pp.tile([128, 2, 30, 30], BF16, tag="x3buf")
              nc.gpsimd.memset(x3buf[:], 0.0)
              x4T = pp.tile([112, 7, 32, 16], F16, tag="x4T")

              # ------------------------------------------------ pools for main pipe
              ps = rctx.enter_context(tc.tile_pool(name="psum", bufs=4, space="PSUM"))
              hp = rctx.enter_context(tc.tile_pool(name="hbuf", bufs=12))
              colp = rctx.enter_context(tc.tile_pool(name="col", bufs=4))
              x4p = rctx.enter_context(tc.tile_pool(name="x4", bufs=2))
              smp = rctx.enter_context(tc.tile_pool(name="small", bufs=2))

              def win(buf, slot, di, dj, h):
                  # [128, 14, 28] shifted window of padded 30x30 map, output half h
                  return buf[:, slot, h * 14 + di:h * 14 + di + 14, dj:dj + 28]

              for ci in range(4):  # sample chunks of SC=8
                  for si in range(SC):
                      s = ci * SC + si
                      slot = s % NSF
                      # features f32 -> bf16 stationary via casting SWDGE DMA
                      nc.gpsimd.dma_start(
                          out=fbuf[:, slot, :, 0:49],
                          in_=features[s].rearrange("(k c) h w -> c k (h w)", k=4),
                      )
                      # stage 0 bf16 col-tiled: even group cols 0-63, odd 64-127
                      psH = ps.tile([128, 2, 512], F32, tag="ps")
                      groups = ((0, ct["c_w1fe"]), (64, ct["c_w1fo"]))
                      for kb in range(4):
                          for tp, wsrc in groups:
                              nc.tensor.matmul(
                                  psH[tp:tp + 64, 0, :],
                                  fbuf[:, slot, kb, :],
                                  wsrc[:, kb, :],
                                  start=(kb == 0), stop=(kb == 3),
                                  tile_position=(0, tp),
                              )
                      for ki in range(2):
                          for tp in (0, 64):
                              kb = ki if tp == 0 else ki + 2
                              nc.tensor.matmul(
                                  psH[tp:tp + 64, 1, 0:128],
                                  fbuf[:, slot, kb, :],
                                  ct["c_w1f8"][:, kb, :],
                                  start=(ki == 0), stop=(ki == 1),
                                  tile_position=(0, tp),
                              )
                      Hs = hp.tile([128, 640], BF16, tag="H")
                      nc.vector.tensor_copy(Hs[:, 0:512], psH[:, 0, :])
                      nc.vector.tensor_copy(Hs[:, 512:640], psH[:, 1, 0:128])

                      # im2col rows (di, ch) = full-width padded row blocks; the dj
                      # shift happens in the matmul rhs AP.  3 contiguous DMAs.
                      col30 = colp.tile([12, 28, 30], F32R, tag="col30")
                      for di in range(3):
                          nc.sync.dma_start(
                              out=col30[4 * di:4 * di + 4],
                              in_=scratch[s, :, di:di + 28, :],
                          )

                      # dj (im2col) matmuls open the group so the PSUM slot is not
                      # claimed until col30 (scratch round-trip) is ready.
                      ps1 = ps.tile([128, 2, 512], F32, tag="ps")
                      for h in range(2):
                          o_ap = ps1[:, h, 0:392]
                          for dj in range(3):
                              nc.tensor.matmul(
                                  o_ap.rearrange("p (a b) -> p a b", a=14),
                                  (ct["c_w1ic3"][:, dj, :]),
                                  (col30[:, h * 14:h * 14 + 14, dj:dj + 28]),
                                  start=(dj == 0), stop=False,
                              )
                          for t in range(5):
                              nc.tensor.matmul(
                                  o_ap,
                                  (Hs[:, 128 * t:128 * (t + 1)]),
                                  (ct["c_uall"][:, t, h * 392:(h + 1) * 392]),
                                  start=False, stop=(t == 4),
                              )
                      for h in range(2):
                          nc.scalar.activation(
                              x1buf[:, si, 1 + 14 * h:15 + 14 * h, 1:29],
                              ps1[:, h, 0:392].rearrange("p (a b) -> p a b", a=14),
                              ACTF.Relu, bias=ct["c_b1"][:], scale=1.0)

                  # conv2: pairs, col-tiled 2 samples
                  for p in range(4):
                      ps2 = ps.tile([128, 2, 512], F32, tag="ps")
                      for h in range(2):
                          for t in range(9):
                              di, dj = divmod(t, 3)
                              for tp, slot in ((0, 2 * p), (64, 2 * p + 1)):
                                  nc.tensor.matmul(
                                      ps2[tp:tp + 64, h, 0:392],
                                      (ct["c_w2"][:, t, :]),
                                      (win(x1buf, slot, di, dj, h)),
                                      start=(t == 0), stop=(t == 8),
                                      tile_position=(0, tp),
                                  )
                      for h in range(2):
                          nc.scalar.activation(
                              x2buf[:, p, 1 + 14 * h:15 + 14 * h, 1:29],
                              ps2[:, h, 0:392].rearrange("p (a b) -> p a b", a=14),
                              ACTF.Relu, bias=ct["c_b2"][:], scale=1.0)

                  # conv3: quads (block-diag 2-sample K), col-tiled 2 pairs
                  for q in range(2):
                      ps3 = ps.tile([128, 2, 512], F32, tag="ps")
                      for h in range(2):
                          for t in range(9):
                              di, dj = divmod(t, 3)
                              for tp, slot in ((0, 2 * q), (64, 2 * q + 1)):
                                  nc.tensor.matmul(
                                      ps3[tp:tp + 64, h, 0:392],
                                      (ct["c_w3"][:, t, :]),
                                      (win(x2buf, slot, di, dj, h)),
                                      start=(t == 0), stop=(t == 8),
                                      tile_position=(0, tp),
                                  )
                      for h in range(2):
                          nc.scalar.activation(
                              x3buf[:, q, 1 + 14 * h:15 + 14 * h, 1:29],
                              ps3[:, h, 0:392].rearrange("p (a b) -> p a b", a=14),
                              ACTF.Relu, bias=ct["c_b3"][:], scale=1.0)

                  # conv4: one octet (block-diag 4-sample K), col-tiled 2 quads
                  ps4 = ps.tile([128, 2, 512], F32, tag="ps")
                  for h in range(2):
                      for t in range(9):
                          di, dj = divmod(t, 3)
                          for tp, slot in ((0, 0), (64, 1)):
                              nc.tensor.matmul(
                                  ps4[tp:tp + 64, h, 0:392],
                                  (ct["c_w4"][:, t, :]),
                                  (win(x3buf, slot, di, dj, h)),
                                  start=(t == 0), stop=(t == 8),
                                  tile_position=(0, tp),
                              )
                  x4t = x4p.tile([128, 784], F32, tag="x4")
                  for h in range(2):
                      nc.scalar.activation(x4t[:, h * 392:(h + 1) * 392],
                                           ps4[:, h, 0:392], ACTF.Relu,
                                           bias=ct["c_b4"][:], scale=1.0)

                  # transpose conv4 output: [64=(4s,16ch), 112ij] -> [112, (4s,16ch)]
                  for h2 in range(2):
                      g = ci * 2 + h2
                      for c in range(7):
                          tr = ps.tile([112, 4, 16], F32, tag="ps")
                          nc.tensor.transpose(
                              tr[:],
                              x4t[64 * h2:64 * h2 + 64, 112 * c:112 * (c + 1)],
                              ct["c_ident"][64 * h2:64 * h2 + 64, 0:64],
                          )
                          nc.vector.tensor_copy(x4T[:, c, 4 * g:4 * g + 4, :], tr[:])

                  # stream fw1 in behind the compute (needed only at fc1)
                  for c in ([2 * ci, 2 * ci + 1] if ci < 3 else [6]):
                      nc.sync.dma_start(out=c_fw1_t[:, c], in_=cst["c_fw1"][:, c])

              # ------------------------------------------------ fc1 / fc2
              psF = ps.tile([32, 256], F32, tag="ps")
              nc.tensor.matmul(psF[:], (ones32[:]), (ct["c_fb1"][:]),
                               start=True, stop=False)
              for c in range(7):
                  for ch in range(16):
                      nc.tensor.matmul(
                          psF[:],
                          x4T[:, c, :, ch],
                          c_fw1_t[:, c, ch, :],
                          start=False, stop=(c == 6 and ch == 15),
                      )
              x5 = smp.tile([32, 256], F32, tag="x5")
              nc.scalar.activation(x5[:], psF[:], ACTF.Relu)

              x5T = smp.tile([128, 2, 32], F32R, tag="x5T")
              for kb in range(2):
                  trF = ps.tile([128, 32], F32, tag="ps")
                  nc.tensor.transpose(trF[:], x5[:, 128 * kb:128 * (kb + 1)],
                                      ct["c_ident"][0:32, 0:32])
                  nc.vector.tensor_copy(x5T[:, kb, :], trF[:])

              psG = ps.tile([32, 24], F32, tag="ps")
              nc.tensor.matmul(psG[:], (ones32[:]), (ct["c_fb2"][:]),
                               start=True, stop=False)
              nc.tensor.matmul(psG[:], (x5T[:, 0, :]), (ct["c_fw2"][:, 0, :]),
                               start=False, stop=False)
              nc.tensor.matmul(psG[:], (x5T[:, 1, :]), (ct["c_fw2"][:, 1, :]),
                               start=False, stop=True)
              osb = smp.tile([32, 24], F32, tag="osb")
              nc.scalar.copy(osb[:, 0:2], psG[:, 0:2])
              nc.scalar.activation(osb[:, 2:4], psG[:, 2:4], ACTF.Sigmoid)
              nc.scalar.copy(osb[:, 4:24], psG[:, 4:24])
              nc.sync.dma_start(out=out_d, in_=osb[:])

    nc.compile()
    return nc


# ------------------------------------------------------------------ entry

_CACHE = {}


def _get_nc(reps=1):
    if reps not in _CACHE:
        _CACHE[reps] = build_nc(reps)
    return _CACHE[reps]


def make_in_maps(**inputs):
    consts = _build_consts(
        inputs["w1"], inputs["b1"], inputs["w2"], inputs["b2"],
        inputs["w3"], inputs["b3"], inputs["w4"], inputs["b4"],
        inputs["fw1"], inputs["fb1"], inputs["fw2"], inputs["fb2"],
    )
    image = np.ascontiguousarray(np.asarray(inputs["image"], np.float32))
    features = np.ascontiguousarray(np.asarray(inputs["features"], np.float32))
    centers = np.ascontiguousarray(np.asarray(inputs["centers"], np.float32))
    in_maps = []
    for i in range(NCORES):
        sl = slice(i * BL, (i + 1) * BL)
        m = {
            "image": np.ascontiguousarray(image[sl]),
            "features": np.ascontiguousarray(features[sl]),
            "centers": np.ascontiguousarray(centers[sl]),
        }
        m.update(consts)
        in_maps.append(m)
    return in_maps


def kernel(**inputs):
    nc = _get_nc()
    in_maps = make_in_maps(**inputs)
    res = run_bass_kernel_spmd(nc, in_maps, core_ids=list(range(NCORES)))
    out = np.concatenate([res.results[i]["out"] for i in range(NCORES)], axis=0)
    return out.astype(np.float32)

